# revision 4
# baseline (speedup 1.0000x reference)
"""LRU (complex diagonal linear recurrence, fwd+bwd) on 8 TRN2 NeuronCores — v3.

Sequence-parallel over T (TC=2048/core). Per core, per state-tile nt:
  BuT = B_norm @ x^T                                     (PE, fp16 matmuls)
  pre-rotation shared between directions (symmetry trick, packed [re|im]):
    prod1 = [cos|cos] (.) [bu_re|bu_im]   prod2 = [sin|sin] (.) [bu_im|bu_re]
    wf_re = p1_lo + p2_lo   wf_im = p1_hi - p2_hi     (fwd scan input)
    qr_re = rev(p1_lo - p2_lo)  qr_im = rev(p1_hi + p2_hi)  (bwd, reversed)
  4 real first-order scans with multiplier r            (Pool engine, DVE aids)
  carry exchange: one small AllGather PER nt of the raw chunk-end scan
  states, so carries unlock while later tiles still scan; all phase
  constants are folded into host-precomputed packed weights W4; one TT +
  one segmented reduce per nt recovers complex carries for both dirs.
  correction: v += rpow (.) chv, in place (fwd pairs on Pool via STT,
  bwd pairs as Act-mul + DVE-add)
  post-rotation STRIPED in 1024-column blocks so each output-projection
  PSUM chain completes within one stripe (no end-of-kernel PE tail):
    s_f = e^{+i theta tau} v2_f     (in place over v)
    s_b = e^{-i theta (TC-1-rho)} v2_b  (into s_b tiles, written reversed)
  y^T = C-projections (fp16 matmuls, PE).  D (.) x added on HOST.
"""

import numpy as np
from contextlib import ExitStack

import concourse.bass as bass
import concourse.tile as tile
from concourse import bacc, mybir
from concourse.bass_utils import run_bass_kernel_spmd

NCORES = 8
T, N, H = 16384, 512, 512
TC = T // NCORES          # 2048 timesteps per core
NT = N // 128             # 4 partition tiles of the state dim
HT = H // 128             # 4 partition tiles of the channel dim
KH = H // 128             # contraction subtiles for Bu matmul
LB = 1024                 # output stripe width
NL = TC // LB
F16 = mybir.dt.float16
F32 = mybir.dt.float32
MUL = mybir.AluOpType.mult
ADD = mybir.AluOpType.add
SUB = mybir.AluOpType.subtract
COPY = mybir.ActivationFunctionType.Copy

_CACHE = {}


def _dup2(ap_2d, w):
    """(128, w) AP -> (128, 2, w) with the row repeated twice (step-0 dup)."""
    return ap_2d.rearrange("p (c t) -> p c t", c=1).broadcast_to([128, 2, w])


def _build_nc(profile=False, iters=1):
    nc = bacc.Bacc(
        "TRN2", target_bir_lowering=False, debug=False,
        enable_asserts=False, num_devices=1 if profile else NCORES,
    )
    di = lambda n, s, d=F32: nc.dram_tensor(n, s, d, kind="ExternalInput")
    xT_d = di("xT", [H, TC], F16)
    BTre_d = di("BTre", [H, N], F16)
    BTim_d = di("BTim", [H, N], F16)
    cos_d = di("cosT", [N, TC], F16)
    sin_d = di("sinT", [N, TC], F16)
    rpw_d = di("rpow", [N, TC], F16)
    cst_d = di("consts", [N, 8])            # col0 = r
    CT_d = {(d_, c_): di(f"CT{d_}{c_}", [N, H], F16)
            for d_ in "fb" for c_ in "ri"}
    W4_d = di("W4", [N, 64])
    yT_d = nc.dram_tensor("yT", [H, TC], F32, kind="ExternalOutput")
    bin_d = nc.dram_tensor("ccin", [128, 16], F32)
    bout_d = nc.dram_tensor("ccout", [NCORES, 128, 16], F32)

    with tile.TileContext(nc) as tc, ExitStack() as ctx:
        pool = lambda name, bufs: ctx.enter_context(tc.tile_pool(name=name, bufs=bufs))
        p_xT = pool("xT", 4)
        p_b4 = pool("b4", 4)
        p_BT = pool("BT", 8)
        p_CT = pool("CT", 16)
        p_cos = pool("cos", 4)
        p_sin = pool("sin", 4)
        p_rpw = pool("rpw", 2)
        p_cst = pool("cst", 4)
        p_bu = pool("bu", 1)
        p_wq = pool("wq", 4)
        p_v = pool("v", 8)
        p_yo = pool("yo", 2)
        p_sm = pool("sm", 2)
        p_ch = pool("ch", 4)
        p_bups = ctx.enter_context(tc.tile_pool(name="bups", bufs=2, space="PSUM"))
        p_ops = ctx.enter_context(tc.tile_pool(name="ops", bufs=2, space="PSUM"))

        for _iter in range(iters):
            # ---- loads: first tables for nt0/nt1, then matmul operands ----
            xT_sb = []
            for h in range(HT):
                t_ = p_xT.tile([128, TC], F16, tag="xT", name="xt")
                nc.sync.dma_start(t_[:], xT_d[h * 128:(h + 1) * 128, :])
                xT_sb.append(t_)
            BT_sb = {}
            for nm, dd in (("re", BTre_d), ("im", BTim_d)):
                for h in range(HT):
                    t_ = p_BT.tile([128, N], F16, tag="BT", name="bt")
                    nc.sync.dma_start(t_[:], dd[h * 128:(h + 1) * 128, :])
                    BT_sb[(nm, h)] = t_
            tabs1 = {}
            for nt in range(2):
                nsl = slice(nt * 128, (nt + 1) * 128)
                cos_t = p_cos.tile([128, TC], F16, tag="cos", name="c0")
                nc.sync.dma_start(cos_t[:], cos_d[nsl, :])
                sin_t = p_sin.tile([128, TC], F16, tag="sin", name="s0")
                nc.sync.dma_start(sin_t[:], sin_d[nsl, :])
                tabs1[nt] = (cos_t, sin_t)
            cst_sb = []
            for nt in range(NT):
                t_ = p_cst.tile([128, 8], F32, tag="cst", name="cs")
                nc.sync.dma_start(t_[:], cst_d[nt * 128:(nt + 1) * 128, :])
                cst_sb.append(t_)

            # ---- phase 1: Bu matmuls, shared pre-rotations, scans ----
            v_sb = {}
            chv_sb = {}
            epk = p_sm.tile([128, 16], F32, tag="epk", name="ep")      # (nt, dir) -> packed (128, 2TC) f16 scan outputs
            for nt in range(NT):
                nsl = slice(nt * 128, (nt + 1) * 128)
                if nt in tabs1:
                    cos_t, sin_t = tabs1[nt]
                else:
                    cos_t = p_cos.tile([128, TC], F16, tag="cos", name="c1")
                    nc.sync.dma_start(cos_t[:], cos_d[nsl, :])
                    sin_t = p_sin.tile([128, TC], F16, tag="sin", name="s1")
                    nc.sync.dma_start(sin_t[:], sin_d[nsl, :])
                    tabs1[nt] = (cos_t, sin_t)
                bu = p_bu.tile([128, 2 * TC], F16, tag="bu", name="bu")
                for ci, nm in enumerate(("re", "im")):
                    for half in range(2):
                        ps = p_bups.tile([128, TC // 2], F32, tag="bups", name="ps")
                        for lc in range(2):
                            sl = slice(half * 1024 + lc * 512, half * 1024 + (lc + 1) * 512)
                            psl = slice(lc * 512, (lc + 1) * 512)
                            for kh in range(KH):
                                nc.tensor.matmul(
                                    ps[:, psl],
                                    BT_sb[(nm, kh)][:, nsl],
                                    xT_sb[kh][:, sl],
                                    start=(kh == 0), stop=(kh == KH - 1),
                                )
                        nc.scalar.copy(
                            bu[:, ci * TC + half * 1024: ci * TC + (half + 1) * 1024],
                            ps[:])
                bu3 = bu[:].rearrange("p (c t) -> p c t", c=2)
                bsw = bu3[:, ::-1, :]
                prod1 = p_wq.tile([128, 2 * TC], F16, tag="wq", name="p1")
                qr = p_wq.tile([128, 2 * TC], F16, tag="wq", name="qr")
                p13 = prod1[:].rearrange("p (c t) -> p c t", c=2)
                p2l = p_b4.tile([128, TC], F16, tag="b4", name="2l")
                p2h = p_b4.tile([128, TC], F16, tag="b4", name="2h")
                nc.vector.tensor_tensor(p13, _dup2(cos_t[:], TC), bu3, MUL)
                nc.vector.tensor_tensor(p2l[:], sin_t[:], bu[:, TC:2 * TC], MUL)
                nc.vector.tensor_tensor(p2h[:], sin_t[:], bu[:, 0:TC], MUL)
                # qr = bwd scan input, stored time-reversed
                nc.vector.tensor_tensor(qr[:, 0:TC][:, ::-1],
                                        prod1[:, 0:TC], p2l[:], SUB)
                nc.vector.tensor_tensor(qr[:, TC:2 * TC][:, ::-1],
                                        prod1[:, TC:2 * TC], p2h[:], ADD)
                # wf in place over prod1
                nc.vector.tensor_tensor(prod1[:, 0:TC],
                                        prod1[:, 0:TC], p2l[:], ADD)
                nc.vector.tensor_tensor(prod1[:, TC:2 * TC],
                                        prod1[:, TC:2 * TC], p2h[:], SUB)
                wf = prod1
                rbc = cst_sb[nt][:, 0:1].broadcast_to([128, TC])
                for di_, (d_, src) in enumerate((("f", wf), ("b", qr))):
                    v = p_v.tile([128, 2 * TC], F16, tag="v", name="v")
                    eng = nc.vector
                    eng.tensor_tensor_scan(
                        v[:, 0:TC], rbc, src[:, 0:TC], 0.0, MUL, ADD)
                    eng.tensor_tensor_scan(
                        v[:, TC:2 * TC], rbc, src[:, TC:2 * TC], 0.0, MUL, ADD)
                    v_sb[(nt, d_)] = v
                    nc.scalar.copy(epk[:, (0 if d_ == "f" else 8) + nt * 2:(0 if d_ == "f" else 8) + nt * 2 + 2], v[:, TC - 1::TC])

            # ---- single carry exchange of all raw end states ----
            nc.sync.dma_start(bin_d[:, :], epk[:])
            if profile:
                for j in range(NCORES):
                    nc.sync.dma_start(bout_d.ap()[j, :, :], bin_d[:, :])
            else:
                nc.gpsimd.collective_compute(
                    "AllGather", mybir.AluOpType.bypass,
                    replica_groups=[list(range(NCORES))],
                    ins=[bin_d.ap().opt()], outs=[bout_d.ap().opt()],
                )
            for nt in range(NT):
                nsl = slice(nt * 128, (nt + 1) * 128)
                eg = p_sm.tile([128, 64], F32, tag="eg", name="eg")
                for quad in range(4):
                    c0 = nt * 2 if quad < 2 else 8 + nt * 2
                    nc.sync.dma_start(
                        eg[:, quad * 16:(quad + 1) * 16].rearrange("p (j c) -> p j c", c=2),
                        bout_d.ap()[:, :, c0:c0 + 2].rearrange("j p c -> p j c"))
                w4_t = p_sm.tile([128, 64], F32, tag="w4", name="w4")
                nc.sync.dma_start(w4_t[:], W4_d[nsl, :])
                pr = p_sm.tile([128, 64], F32, tag="pr", name="pr")
                nc.vector.tensor_tensor(pr[:], w4_t[:], eg[:], MUL)
                chv = p_ch.tile([128, 4], F32, tag="chv", name="ch")
                nc.vector.tensor_reduce(
                    chv[:].rearrange("p (s o) -> p s o", o=1),
                    pr[:].rearrange("p (s j) -> p s j", s=4),
                    mybir.AxisListType.X, ADD)
                chv_sb[nt] = chv

            # prefetch correction tables while phase 1 drains
            rpw_sb = {}
            for nt in range(2):
                rpw_t = p_rpw.tile([128, TC], F16, tag="rpw", name="rq")
                nc.sync.dma_start(rpw_t[:], rpw_d[nt * 128:(nt + 1) * 128, :])
                rpw_sb[nt] = rpw_t

            # ---- phase 2a: per nt full-width corrections ----
            for nt in range(NT):
                nsl = slice(nt * 128, (nt + 1) * 128)
                chv = chv_sb[nt]
                if nt in rpw_sb:
                    rpw_t = rpw_sb[nt]
                else:
                    rpw_t = p_rpw.tile([128, TC], F16, tag="rpw", name="rp")
                    nc.sync.dma_start(rpw_t[:], rpw_d[nsl, :])
                v = v_sb[(nt, "f")]
                vb = v_sb[(nt, "b")]
                for ci, (vv, sc) in enumerate(((v, 0), (v, 1), (vb, 2), (vb, 3))):
                    t_ = p_b4.tile([128, TC], F16, tag="b4", name="t4")
                    nc.scalar.activation(t_[:], rpw_t[:], COPY,
                                         scale=chv[:, sc:sc + 1])
                    half = slice(0, TC) if ci % 2 == 0 else slice(TC, 2 * TC)
                    nc.vector.tensor_tensor(vv[:, half], t_[:], vv[:, half], ADD)

            # ---- late loads (needed only from mid-kernel on) ----
            CT_sb = {}
            for key, dd in CT_d.items():
                for nt in range(NT):
                    t_ = p_CT.tile([128, H], F16, tag="CT", name="ct")
                    nc.sync.dma_start(t_[:], dd[nt * 128:(nt + 1) * 128, :])
                    CT_sb[key + (nt,)] = t_
            rpw_sb = {}

            # ---- phase 2b: lc-major striped post-rot + output projections ----
            sb_sb = {nt: p_wq.tile([128, 2 * TC], F16, tag="wq", name="sb")
                     for nt in range(NT)}
            for lc in range(NL):
                a, b = lc * LB, (lc + 1) * LB
                ra, rb = TC - b, TC - a
                lsl = slice(a, b)
                hsl_i = slice(TC + a, TC + b)
                rsl = slice(ra, rb)
                for nt in range(NT):
                    cos_t, sin_t = tabs1[nt]
                    v = v_sb[(nt, "f")]
                    vb = v_sb[(nt, "b")]
                    sb = sb_sb[nt]
                    v3 = v[:].rearrange("p (c t) -> p c t", c=2)
                    vsw = v3[:, ::-1, :]
                    vb3 = vb[:].rearrange("p (c t) -> p c t", c=2)
                    vbsw = vb3[:, ::-1, :]
                    pp = p_b4.tile([128, 2 * LB], F16, tag="b4", name="pp")
                    zz = p_b4.tile([128, 2 * LB], F16, tag="b4", name="zz")
                    pp3 = pp[:].rearrange("p (c t) -> p c t", c=2)
                    zz3 = zz[:].rearrange("p (c t) -> p c t", c=2)
                    nc.vector.tensor_tensor(
                        pp3, _dup2(cos_t[:], TC)[:, :, lsl], v3[:, :, lsl], MUL)
                    nc.vector.tensor_tensor(
                        zz3, _dup2(sin_t[:], TC)[:, :, lsl], vsw[:, :, lsl], MUL)
                    nc.vector.tensor_tensor(v[:, lsl], pp[:, 0:LB], zz[:, 0:LB], SUB)
                    nc.vector.tensor_tensor(v[:, hsl_i], pp[:, LB:2 * LB],
                                            zz[:, LB:2 * LB], ADD)
                    # bwd products on the (otherwise idle) Pool engine
                    ppb = p_b4.tile([128, 2 * LB], F16, tag="b4", name="pb")
                    zzb = p_b4.tile([128, 2 * LB], F16, tag="b4", name="zb")
                    ppb3 = ppb[:].rearrange("p (c t) -> p c t", c=2)
                    zzb3 = zzb[:].rearrange("p (c t) -> p c t", c=2)
                    nc.vector.tensor_tensor(
                        ppb3, _dup2(cos_t[:, ::-1], TC)[:, :, rsl], vb3[:, :, rsl], MUL)
                    nc.vector.tensor_tensor(
                        zzb3, _dup2(sin_t[:, ::-1], TC)[:, :, rsl], vbsw[:, :, rsl], MUL)
                    nc.vector.tensor_tensor(sb[:, lsl][:, ::-1],
                                            ppb[:, 0:LB], zzb[:, 0:LB], ADD)
                    nc.vector.tensor_tensor(sb[:, hsl_i][:, ::-1],
                                            ppb[:, LB:2 * LB], zzb[:, LB:2 * LB], SUB)
                for ht in range(HT):
                    hsl = slice(ht * 128, (ht + 1) * 128)
                    # FD=512 sub-chains: one matmul must fit one PSUM bank
                    for half in range(2):
                        aa = a + half * 512
                        l5 = slice(aa, aa + 512)
                        h5 = slice(TC + aa, TC + aa + 512)
                        ps = p_ops.tile([128, 512], F32, tag="ops", name="op")
                        groups = []
                        for nt in range(NT):
                            groups.append((CT_sb[("f", "r", nt)], v_sb[(nt, "f")], l5))
                            groups.append((CT_sb[("f", "i", nt)], v_sb[(nt, "f")], h5))
                            groups.append((CT_sb[("b", "r", nt)], sb_sb[nt], l5))
                            groups.append((CT_sb[("b", "i", nt)], sb_sb[nt], h5))
                        for gi, (ct, sv, sl_) in enumerate(groups):
                            nc.tensor.matmul(
                                ps[:], ct[:, hsl], sv[:, sl_],
                                start=(gi == 0), stop=(gi == len(groups) - 1),
                            )
                        yo = p_yo.tile([128, 512], F32, tag="yo", name="yo")
                        nc.scalar.copy(yo[:], ps[:])
                        nc.sync.dma_start(yT_d[hsl, l5], yo[:])

    nc.compile()
    return nc


def _host_prep(x, theta_log, nu_log, B_re, B_im, C_re, C_im, C_re2, C_im2, D):
    f64 = np.float64
    theta = np.exp(theta_log.astype(f64))
    r = np.exp(-np.exp(nu_log.astype(f64)))
    gamma = np.sqrt(1.0 - r ** 2)
    Bn = (B_re.astype(f64) + 1j * B_im.astype(f64)) * gamma[:, None]
    Lam = r * np.exp(1j * theta)
    tau = np.arange(TC, dtype=f64)
    cosT = np.cos(theta[:, None] * tau).astype(np.float16)
    sinT = np.sin(theta[:, None] * tau).astype(np.float16)
    rpow = (r[:, None] ** (tau + 1)).astype(np.float16)
    consts = np.zeros((N, 8), np.float32)
    consts[:, 0] = r
    xT = np.ascontiguousarray(x.T.astype(np.float16))        # (H, T)
    BTre = np.ascontiguousarray(Bn.real.T.astype(np.float16))
    BTim = np.ascontiguousarray(Bn.imag.T.astype(np.float16))
    C1 = C_re.astype(f64) + 1j * C_im.astype(f64)
    C2 = C_re2.astype(f64) + 1j * C_im2.astype(f64)
    CT = {
        ("f", "r"): C1.real.T, ("f", "i"): -C1.imag.T,
        ("b", "r"): C2.real.T, ("b", "i"): -C2.imag.T,
    }
    CT = {k: np.ascontiguousarray(v.astype(np.float16)) for k, v in CT.items()}
    LamTC = Lam ** TC
    phase = np.exp(1j * theta * TC)
    W4 = []
    for k in range(NCORES):
        wf = np.zeros((N, NCORES), np.complex128)
        wb = np.zeros((N, NCORES), np.complex128)
        for j in range(k):
            wf[:, j] = phase * LamTC ** (k - 1 - j)
        for j in range(k + 1, NCORES):
            wb[:, j] = phase * LamTC ** (j - k - 1)

        def inter(a_, b_):
            return np.stack([a_, b_], axis=-1).reshape(N, 2 * NCORES)
        w4 = np.concatenate(
            [inter(wf.real, -wf.imag), inter(wf.imag, wf.real),
             inter(wb.real, -wb.imag), inter(wb.imag, wb.real)],
            axis=1).astype(np.float32)
        W4.append(np.ascontiguousarray(w4))
    Dx = (D.astype(f64)[None, :] * x.astype(f64)).astype(np.float32)
    return xT, BTre, BTim, cosT, sinT, rpow, consts, CT, W4, Dx


def make_in_maps(inputs):
    xT, BTre, BTim, cosT, sinT, rpow, consts, CT, W4, Dx = _host_prep(**inputs)
    in_maps = []
    for k in range(NCORES):
        in_maps.append({
            "xT": np.ascontiguousarray(xT[:, k * TC:(k + 1) * TC]),
            "BTre": BTre, "BTim": BTim,
            "cosT": cosT, "sinT": sinT, "rpow": rpow, "consts": consts,
            "CTfr": CT[("f", "r")], "CTfi": CT[("f", "i")],
            "CTbr": CT[("b", "r")], "CTbi": CT[("b", "i")],
            "W4": W4[k],
        })
    return in_maps, Dx


def kernel(**inputs):
    if "nc" not in _CACHE:
        _CACHE["nc"] = _build_nc()
    nc = _CACHE["nc"]
    in_maps, Dx = make_in_maps(inputs)
    res = run_bass_kernel_spmd(nc, in_maps, core_ids=list(range(NCORES)))
    yT = np.concatenate([res.results[k]["yT"] for k in range(NCORES)], axis=1)
    return (np.ascontiguousarray(yT.T) + Dx).astype(np.float32)


# revision 5
# speedup vs baseline: 1.0246x; 1.0246x over previous
"""LRU (complex diagonal linear recurrence, fwd+bwd) on 8 TRN2 NeuronCores — v3.

Sequence-parallel over T (TC=2048/core). Per core, per state-tile nt:
  BuT = B_norm @ x^T                                     (PE, fp16 matmuls)
  pre-rotation shared between directions (symmetry trick, packed [re|im]):
    prod1 = [cos|cos] (.) [bu_re|bu_im]   prod2 = [sin|sin] (.) [bu_im|bu_re]
    wf_re = p1_lo + p2_lo   wf_im = p1_hi - p2_hi     (fwd scan input)
    qr_re = rev(p1_lo - p2_lo)  qr_im = rev(p1_hi + p2_hi)  (bwd, reversed)
  4 real first-order scans with multiplier r (DVE; Pool/GpSimd cannot run
  elementwise/scan ops in this NEFF lowering -- compile-time ISA check)
  carry exchange: one small AllGather PER nt of the raw chunk-end scan
  states, so carries unlock while later tiles still scan; all phase
  constants are folded into host-precomputed packed weights W4; one TT +
  one segmented reduce per nt recovers complex carries for both dirs.
  correction: v += rpow (.) chv, in place (Act computes the product with a
  per-partition scale, DVE adds)
  post-rotation STRIPED in 1024-column blocks so output-projection PSUM
  chains (FD=512 each; one matmul must fit one PSUM bank) complete within
  one stripe -- no end-of-kernel PE tail:
    s_f = e^{+i theta tau} v2_f     (in place over v)
    s_b = e^{-i theta (TC-1-rho)} v2_b  (into s_b tiles, written reversed)
  y^T = C-projections (fp16 matmuls, PE).  D (.) x added on HOST.
"""

import numpy as np
from contextlib import ExitStack

import concourse.bass as bass
import concourse.tile as tile
from concourse import bacc, mybir
from concourse.bass_utils import run_bass_kernel_spmd

NCORES = 8
T, N, H = 16384, 512, 512
TC = T // NCORES          # 2048 timesteps per core
NT = N // 128             # 4 partition tiles of the state dim
HT = H // 128             # 4 partition tiles of the channel dim
KH = H // 128             # contraction subtiles for Bu matmul
LB = 1024                 # output stripe width
NL = TC // LB
F16 = mybir.dt.float16
F32 = mybir.dt.float32
MUL = mybir.AluOpType.mult
ADD = mybir.AluOpType.add
SUB = mybir.AluOpType.subtract
COPY = mybir.ActivationFunctionType.Copy

_CACHE = {}


def _dup2(ap_2d, w):
    """(128, w) AP -> (128, 2, w) with the row repeated twice (step-0 dup)."""
    return ap_2d.rearrange("p (c t) -> p c t", c=1).broadcast_to([128, 2, w])


def _build_nc(profile=False, iters=1):
    nc = bacc.Bacc(
        "TRN2", target_bir_lowering=False, debug=False,
        enable_asserts=False, num_devices=1 if profile else NCORES,
    )
    di = lambda n, s, d=F32: nc.dram_tensor(n, s, d, kind="ExternalInput")
    xT_d = di("xT", [H, TC], F16)
    BTre_d = di("BTre", [H, N], F16)
    BTim_d = di("BTim", [H, N], F16)
    cos_d = di("cosT", [N, TC], F16)
    sin_d = di("sinT", [N, TC], F16)
    rpw_d = di("rpow", [N, TC], F16)
    cst_d = di("consts", [N, 8])            # col0 = r
    CT_d = {(d_, c_): di(f"CT{d_}{c_}", [N, H], F16)
            for d_ in "fb" for c_ in "ri"}
    W4_d = di("W4", [N, 64])
    yT_d = nc.dram_tensor("yT", [H, TC], F32, kind="ExternalOutput")
    bin_d = nc.dram_tensor("ccin", [128, 16], F32)
    bout_d = nc.dram_tensor("ccout", [NCORES, 128, 16], F32)

    with tile.TileContext(nc) as tc, ExitStack() as ctx:
        pool = lambda name, bufs: ctx.enter_context(tc.tile_pool(name=name, bufs=bufs))
        p_xT = pool("xT", 4)
        p_b4 = pool("b4", 4)
        p_BT = pool("BT", 8)
        p_CT = pool("CT", 16)
        p_cos = pool("cos", 4)
        p_sin = pool("sin", 4)
        p_rpw = pool("rpw", 2)
        p_cst = pool("cst", 4)
        p_bu = pool("bu", 1)
        p_wq = pool("wq", 4)
        p_v = pool("v", 8)
        p_yo = pool("yo", 2)
        p_sm = pool("sm", 2)
        p_ch = pool("ch", 4)
        p_bups = ctx.enter_context(tc.tile_pool(name="bups", bufs=2, space="PSUM"))
        p_ops = ctx.enter_context(tc.tile_pool(name="ops", bufs=2, space="PSUM"))

        for _iter in range(iters):
            # ---- loads: first tables for nt0/nt1, then matmul operands ----
            xT_sb = []
            for h in range(HT):
                t_ = p_xT.tile([128, TC], F16, tag="xT", name="xt")
                nc.sync.dma_start(t_[:], xT_d[h * 128:(h + 1) * 128, :])
                xT_sb.append(t_)
            BT_sb = {}
            for nm, dd in (("re", BTre_d), ("im", BTim_d)):
                for h in range(HT):
                    t_ = p_BT.tile([128, N], F16, tag="BT", name="bt")
                    nc.sync.dma_start(t_[:], dd[h * 128:(h + 1) * 128, :])
                    BT_sb[(nm, h)] = t_
            tabs1 = {}
            for nt in range(2):
                nsl = slice(nt * 128, (nt + 1) * 128)
                cos_t = p_cos.tile([128, TC], F16, tag="cos", name="c0")
                nc.sync.dma_start(cos_t[:], cos_d[nsl, :])
                sin_t = p_sin.tile([128, TC], F16, tag="sin", name="s0")
                nc.sync.dma_start(sin_t[:], sin_d[nsl, :])
                tabs1[nt] = (cos_t, sin_t)
            cst_sb = []
            for nt in range(NT):
                t_ = p_cst.tile([128, 8], F32, tag="cst", name="cs")
                nc.sync.dma_start(t_[:], cst_d[nt * 128:(nt + 1) * 128, :])
                cst_sb.append(t_)

            # ---- phase 1: Bu matmuls, shared pre-rotations, scans ----
            v_sb = {}
            chv_sb = {}
            epk = p_sm.tile([128, 16], F32, tag="epk", name="ep")      # (nt, dir) -> packed (128, 2TC) f16 scan outputs
            for nt in range(NT):
                nsl = slice(nt * 128, (nt + 1) * 128)
                if nt in tabs1:
                    cos_t, sin_t = tabs1[nt]
                else:
                    cos_t = p_cos.tile([128, TC], F16, tag="cos", name="c1")
                    nc.sync.dma_start(cos_t[:], cos_d[nsl, :])
                    sin_t = p_sin.tile([128, TC], F16, tag="sin", name="s1")
                    nc.sync.dma_start(sin_t[:], sin_d[nsl, :])
                    tabs1[nt] = (cos_t, sin_t)
                bu = p_bu.tile([128, 2 * TC], F16, tag="bu", name="bu")
                for ci, nm in enumerate(("re", "im")):
                    for half in range(2):
                        ps = p_bups.tile([128, TC // 2], F32, tag="bups", name="ps")
                        for lc in range(2):
                            sl = slice(half * 1024 + lc * 512, half * 1024 + (lc + 1) * 512)
                            psl = slice(lc * 512, (lc + 1) * 512)
                            for kh in range(KH):
                                nc.tensor.matmul(
                                    ps[:, psl],
                                    BT_sb[(nm, kh)][:, nsl],
                                    xT_sb[kh][:, sl],
                                    start=(kh == 0), stop=(kh == KH - 1),
                                )
                        nc.scalar.copy(
                            bu[:, ci * TC + half * 1024: ci * TC + (half + 1) * 1024],
                            ps[:])
                bu3 = bu[:].rearrange("p (c t) -> p c t", c=2)
                bsw = bu3[:, ::-1, :]
                prod1 = p_wq.tile([128, 2 * TC], F16, tag="wq", name="p1")
                qr = p_wq.tile([128, 2 * TC], F16, tag="wq", name="qr")
                p13 = prod1[:].rearrange("p (c t) -> p c t", c=2)
                p2l = p_b4.tile([128, TC], F16, tag="b4", name="2l")
                p2h = p_b4.tile([128, TC], F16, tag="b4", name="2h")
                nc.vector.tensor_tensor(p13, _dup2(cos_t[:], TC), bu3, MUL)
                nc.vector.tensor_tensor(p2l[:], sin_t[:], bu[:, TC:2 * TC], MUL)
                nc.vector.tensor_tensor(p2h[:], sin_t[:], bu[:, 0:TC], MUL)
                # qr = bwd scan input, stored time-reversed
                nc.vector.tensor_tensor(qr[:, 0:TC][:, ::-1],
                                        prod1[:, 0:TC], p2l[:], SUB)
                nc.vector.tensor_tensor(qr[:, TC:2 * TC][:, ::-1],
                                        prod1[:, TC:2 * TC], p2h[:], ADD)
                # wf in place over prod1
                nc.vector.tensor_tensor(prod1[:, 0:TC],
                                        prod1[:, 0:TC], p2l[:], ADD)
                nc.vector.tensor_tensor(prod1[:, TC:2 * TC],
                                        prod1[:, TC:2 * TC], p2h[:], SUB)
                wf = prod1
                rbc = cst_sb[nt][:, 0:1].broadcast_to([128, TC])
                for di_, (d_, src) in enumerate((("f", wf), ("b", qr))):
                    v = p_v.tile([128, 2 * TC], F16, tag="v", name="v")
                    nc.vector.tensor_tensor_scan(
                        v[:, 0:TC], rbc, src[:, 0:TC], 0.0, MUL, ADD)
                    nc.vector.tensor_tensor_scan(
                        v[:, TC:2 * TC], rbc, src[:, TC:2 * TC], 0.0, MUL, ADD)
                    v_sb[(nt, d_)] = v
                    nc.scalar.copy(epk[:, (0 if d_ == "f" else 8) + nt * 2:(0 if d_ == "f" else 8) + nt * 2 + 2], v[:, TC - 1::TC])

            # ---- single carry exchange of all raw end states ----
            nc.sync.dma_start(bin_d[:, :], epk[:])
            if profile:
                for j in range(NCORES):
                    nc.sync.dma_start(bout_d.ap()[j, :, :], bin_d[:, :])
            else:
                nc.gpsimd.collective_compute(
                    "AllGather", mybir.AluOpType.bypass,
                    replica_groups=[list(range(NCORES))],
                    ins=[bin_d.ap().opt()], outs=[bout_d.ap().opt()],
                )
            for nt in range(NT):
                nsl = slice(nt * 128, (nt + 1) * 128)
                eg = p_sm.tile([128, 64], F32, tag="eg", name="eg")
                for quad in range(4):
                    c0 = nt * 2 if quad < 2 else 8 + nt * 2
                    nc.sync.dma_start(
                        eg[:, quad * 16:(quad + 1) * 16].rearrange("p (j c) -> p j c", c=2),
                        bout_d.ap()[:, :, c0:c0 + 2].rearrange("j p c -> p j c"))
                w4_t = p_sm.tile([128, 64], F32, tag="w4", name="w4")
                nc.sync.dma_start(w4_t[:], W4_d[nsl, :])
                pr = p_sm.tile([128, 64], F32, tag="pr", name="pr")
                nc.vector.tensor_tensor(pr[:], w4_t[:], eg[:], MUL)
                chv = p_ch.tile([128, 4], F32, tag="chv", name="ch")
                nc.vector.tensor_reduce(
                    chv[:].rearrange("p (s o) -> p s o", o=1),
                    pr[:].rearrange("p (s j) -> p s j", s=4),
                    mybir.AxisListType.X, ADD)
                chv_sb[nt] = chv

            # prefetch correction tables while phase 1 drains
            rpw_sb = {}
            for nt in range(2):
                rpw_t = p_rpw.tile([128, TC], F16, tag="rpw", name="rq")
                nc.sync.dma_start(rpw_t[:], rpw_d[nt * 128:(nt + 1) * 128, :])
                rpw_sb[nt] = rpw_t

            # ---- phase 2a: per nt full-width corrections ----
            for nt in range(NT):
                nsl = slice(nt * 128, (nt + 1) * 128)
                chv = chv_sb[nt]
                if nt in rpw_sb:
                    rpw_t = rpw_sb[nt]
                else:
                    rpw_t = p_rpw.tile([128, TC], F16, tag="rpw", name="rp")
                    nc.sync.dma_start(rpw_t[:], rpw_d[nsl, :])
                v = v_sb[(nt, "f")]
                vb = v_sb[(nt, "b")]
                for ci, (vv, sc) in enumerate(((v, 0), (v, 1), (vb, 2), (vb, 3))):
                    t_ = p_b4.tile([128, TC], F16, tag="b4", name="t4")
                    nc.scalar.activation(t_[:], rpw_t[:], COPY,
                                         scale=chv[:, sc:sc + 1])
                    half = slice(0, TC) if ci % 2 == 0 else slice(TC, 2 * TC)
                    nc.vector.tensor_tensor(vv[:, half], t_[:], vv[:, half], ADD)

            # ---- late loads (needed only from mid-kernel on) ----
            CT_sb = {}
            for key, dd in CT_d.items():
                for nt in range(NT):
                    t_ = p_CT.tile([128, H], F16, tag="CT", name="ct")
                    nc.sync.dma_start(t_[:], dd[nt * 128:(nt + 1) * 128, :])
                    CT_sb[key + (nt,)] = t_
            rpw_sb = {}

            # ---- phase 2b: lc-major striped post-rot + output projections ----
            sb_sb = {nt: p_wq.tile([128, 2 * TC], F16, tag="wq", name="sb")
                     for nt in range(NT)}
            for lc in range(NL):
                a, b = lc * LB, (lc + 1) * LB
                ra, rb = TC - b, TC - a
                lsl = slice(a, b)
                hsl_i = slice(TC + a, TC + b)
                rsl = slice(ra, rb)
                for nt in range(NT):
                    cos_t, sin_t = tabs1[nt]
                    v = v_sb[(nt, "f")]
                    vb = v_sb[(nt, "b")]
                    sb = sb_sb[nt]
                    v3 = v[:].rearrange("p (c t) -> p c t", c=2)
                    vsw = v3[:, ::-1, :]
                    vb3 = vb[:].rearrange("p (c t) -> p c t", c=2)
                    vbsw = vb3[:, ::-1, :]
                    pp = p_b4.tile([128, 2 * LB], F16, tag="b4", name="pp")
                    zz = p_b4.tile([128, 2 * LB], F16, tag="b4", name="zz")
                    pp3 = pp[:].rearrange("p (c t) -> p c t", c=2)
                    zz3 = zz[:].rearrange("p (c t) -> p c t", c=2)
                    nc.vector.tensor_tensor(
                        pp3, _dup2(cos_t[:], TC)[:, :, lsl], v3[:, :, lsl], MUL)
                    nc.vector.tensor_tensor(
                        zz3, _dup2(sin_t[:], TC)[:, :, lsl], vsw[:, :, lsl], MUL)
                    nc.vector.tensor_tensor(v[:, lsl], pp[:, 0:LB], zz[:, 0:LB], SUB)
                    nc.vector.tensor_tensor(v[:, hsl_i], pp[:, LB:2 * LB],
                                            zz[:, LB:2 * LB], ADD)
                    # bwd products on the (otherwise idle) Pool engine
                    ppb = p_b4.tile([128, 2 * LB], F16, tag="b4", name="pb")
                    zzb = p_b4.tile([128, 2 * LB], F16, tag="b4", name="zb")
                    ppb3 = ppb[:].rearrange("p (c t) -> p c t", c=2)
                    zzb3 = zzb[:].rearrange("p (c t) -> p c t", c=2)
                    nc.vector.tensor_tensor(
                        ppb3, _dup2(cos_t[:, ::-1], TC)[:, :, rsl], vb3[:, :, rsl], MUL)
                    nc.vector.tensor_tensor(
                        zzb3, _dup2(sin_t[:, ::-1], TC)[:, :, rsl], vbsw[:, :, rsl], MUL)
                    nc.vector.tensor_tensor(sb[:, lsl][:, ::-1],
                                            ppb[:, 0:LB], zzb[:, 0:LB], ADD)
                    nc.vector.tensor_tensor(sb[:, hsl_i][:, ::-1],
                                            ppb[:, LB:2 * LB], zzb[:, LB:2 * LB], SUB)
                for ht in range(HT):
                    hsl = slice(ht * 128, (ht + 1) * 128)
                    # FD=512 sub-chains: one matmul must fit one PSUM bank
                    for half in range(2):
                        aa = a + half * 512
                        l5 = slice(aa, aa + 512)
                        h5 = slice(TC + aa, TC + aa + 512)
                        ps = p_ops.tile([128, 512], F32, tag="ops", name="op")
                        groups = []
                        for nt in range(NT):
                            groups.append((CT_sb[("f", "r", nt)], v_sb[(nt, "f")], l5))
                            groups.append((CT_sb[("f", "i", nt)], v_sb[(nt, "f")], h5))
                            groups.append((CT_sb[("b", "r", nt)], sb_sb[nt], l5))
                            groups.append((CT_sb[("b", "i", nt)], sb_sb[nt], h5))
                        for gi, (ct, sv, sl_) in enumerate(groups):
                            nc.tensor.matmul(
                                ps[:], ct[:, hsl], sv[:, sl_],
                                start=(gi == 0), stop=(gi == len(groups) - 1),
                            )
                        yo = p_yo.tile([128, 512], F32, tag="yo", name="yo")
                        nc.scalar.copy(yo[:], ps[:])
                        nc.sync.dma_start(yT_d[hsl, l5], yo[:])

    nc.compile()
    return nc


def _host_prep(x, theta_log, nu_log, B_re, B_im, C_re, C_im, C_re2, C_im2, D):
    f64 = np.float64
    theta = np.exp(theta_log.astype(f64))
    r = np.exp(-np.exp(nu_log.astype(f64)))
    gamma = np.sqrt(1.0 - r ** 2)
    Bn = (B_re.astype(f64) + 1j * B_im.astype(f64)) * gamma[:, None]
    Lam = r * np.exp(1j * theta)
    tau = np.arange(TC, dtype=f64)
    cosT = np.cos(theta[:, None] * tau).astype(np.float16)
    sinT = np.sin(theta[:, None] * tau).astype(np.float16)
    rpow = (r[:, None] ** (tau + 1)).astype(np.float16)
    consts = np.zeros((N, 8), np.float32)
    consts[:, 0] = r
    xT = np.ascontiguousarray(x.T.astype(np.float16))        # (H, T)
    BTre = np.ascontiguousarray(Bn.real.T.astype(np.float16))
    BTim = np.ascontiguousarray(Bn.imag.T.astype(np.float16))
    C1 = C_re.astype(f64) + 1j * C_im.astype(f64)
    C2 = C_re2.astype(f64) + 1j * C_im2.astype(f64)
    CT = {
        ("f", "r"): C1.real.T, ("f", "i"): -C1.imag.T,
        ("b", "r"): C2.real.T, ("b", "i"): -C2.imag.T,
    }
    CT = {k: np.ascontiguousarray(v.astype(np.float16)) for k, v in CT.items()}
    LamTC = Lam ** TC
    phase = np.exp(1j * theta * TC)
    W4 = []
    for k in range(NCORES):
        wf = np.zeros((N, NCORES), np.complex128)
        wb = np.zeros((N, NCORES), np.complex128)
        for j in range(k):
            wf[:, j] = phase * LamTC ** (k - 1 - j)
        for j in range(k + 1, NCORES):
            wb[:, j] = phase * LamTC ** (j - k - 1)

        def inter(a_, b_):
            return np.stack([a_, b_], axis=-1).reshape(N, 2 * NCORES)
        w4 = np.concatenate(
            [inter(wf.real, -wf.imag), inter(wf.imag, wf.real),
             inter(wb.real, -wb.imag), inter(wb.imag, wb.real)],
            axis=1).astype(np.float32)
        W4.append(np.ascontiguousarray(w4))
    Dx = (D.astype(f64)[None, :] * x.astype(f64)).astype(np.float32)
    return xT, BTre, BTim, cosT, sinT, rpow, consts, CT, W4, Dx


def make_in_maps(inputs):
    xT, BTre, BTim, cosT, sinT, rpow, consts, CT, W4, Dx = _host_prep(**inputs)
    in_maps = []
    for k in range(NCORES):
        in_maps.append({
            "xT": np.ascontiguousarray(xT[:, k * TC:(k + 1) * TC]),
            "BTre": BTre, "BTim": BTim,
            "cosT": cosT, "sinT": sinT, "rpow": rpow, "consts": consts,
            "CTfr": CT[("f", "r")], "CTfi": CT[("f", "i")],
            "CTbr": CT[("b", "r")], "CTbi": CT[("b", "i")],
            "W4": W4[k],
        })
    return in_maps, Dx


def kernel(**inputs):
    if "nc" not in _CACHE:
        _CACHE["nc"] = _build_nc()
    nc = _CACHE["nc"]
    in_maps, Dx = make_in_maps(inputs)
    res = run_bass_kernel_spmd(nc, in_maps, core_ids=list(range(NCORES)))
    yT = np.concatenate([res.results[k]["yT"] for k in range(NCORES)], axis=1)
    return (np.ascontiguousarray(yT.T) + Dx).astype(np.float32)
